# revision 1
# baseline (speedup 1.0000x reference)
"""AxialAttention Trainium2 kernel: 8-core SPMD, no collectives.

Sharding: core (b, j) computes height-attention for x[b, :, 64j:64j+64, :]
and width-attention for x[b, 32j:32j+32, :, :]; host sums partial outputs.

Per-phase on-device dataflow (all matmuls bf16, fp32 PSUM):
  xT resident [C=256 (2 part-chunks), tokens=8192]
  qT,kT = W.T @ x   (lhsT = W chunks)            [256, 8192] bf16
  v     = x @ Wv    (lhsT = xT token-tiles) -> v_aug [128, 8*33] per token tile
          (33rd column of ones per head rides the AV matmul -> softmax denom)
  scores sT = k-stationary, 4-head row-group packing (K=32)
  aT = exp(scale * sT)  on ScalarE, PSUM->SBUF bf16
  AV: stationary [v_h | 1] (M=33), 2-head col-group packing -> [ohT_h; denom_h]
  denom rows DMA-compacted -> one DVE reciprocal per half-phase
  recip rows DMA partition-broadcast -> bc tiles; GPSIMD multiply normalizes
  oproj: zero-padded per-pair Wo' (K=128) accumulating in PSUM -> DMA to DRAM
Host: reassemble, add biases, sum height+width partial outputs.
"""

import numpy as np
import ml_dtypes

B, H, W, C = 2, 128, 256, 256
HEADS, D = 8, 32
SCALE = float(D) ** -0.5
WC = W // 4   # 64 w-columns per core (height phase)
HC = H // 4   # 32 h-rows per core (width phase)
NTOK = 8192   # tokens per core per phase
BF16 = ml_dtypes.bfloat16

_compiled = {}


def _build_module():
    import contextlib
    import concourse.bass as bass  # noqa: F401
    from concourse import bacc, mybir
    from concourse.tile import TileContext

    bf = mybir.dt.bfloat16
    f32 = mybir.dt.float32
    Exp = mybir.ActivationFunctionType.Exp
    mult = mybir.AluOpType.mult

    nc = bacc.Bacc("TRN2", target_bir_lowering=False)

    # ---- DRAM I/O ----
    xh = nc.dram_tensor("xh", [2, 128, NTOK], bf, kind="ExternalInput")
    xw = nc.dram_tensor("xw", [2, 128, NTOK], bf, kind="ExternalInput")
    wts = {}
    for ph in ("h", "w"):
        wts[f"wq_{ph}"] = nc.dram_tensor(f"wq_{ph}", [2, 128, 256], bf, kind="ExternalInput")
        wts[f"wk_{ph}"] = nc.dram_tensor(f"wk_{ph}", [2, 128, 256], bf, kind="ExternalInput")
        wts[f"wv_{ph}"] = nc.dram_tensor(f"wv_{ph}", [2, 128, 512], bf, kind="ExternalInput")
        wts[f"wo_{ph}"] = nc.dram_tensor(f"wo_{ph}", [4, 2, 128, 128], bf, kind="ExternalInput")
    out_h = nc.dram_tensor("out_h", [2, 128, WC * 128], f32, kind="ExternalOutput")
    out_w = nc.dram_tensor("out_w", [2, 128, HC * 256], f32, kind="ExternalOutput")

    def phase(tc, ctx, xT_dram, wq_d, wk_d, wv_d, wo_d, out_d, is_width):
        tag = "w" if is_width else "h"
        XBLK = 256 if is_width else 128          # attention span per block
        nblk = HC if is_width else WC            # 32 or 64 blocks
        half = nblk // 2
        AVW = 4 * XBLK                           # av psum width: 4 pairs

        pool = ctx.enter_context(tc.tile_pool(name="persist", bufs=1))
        pb_pool = ctx.enter_context(tc.tile_pool(name="pb", bufs=half))
        work = ctx.enter_context(tc.tile_pool(name="work", bufs=3))
        at_pool = ctx.enter_context(tc.tile_pool(name="at", bufs=3))
        ps_s = ctx.enter_context(tc.tile_pool(name="ps_s", bufs=1, space="PSUM"))
        ps_av = ctx.enter_context(tc.tile_pool(name="ps_av", bufs=1 if is_width else 2, space="PSUM"))
        ps_o = ctx.enter_context(tc.tile_pool(name="ps_o", bufs=2, space="PSUM"))

        # ---- weights + xT to SBUF ----
        def load(dram_ap, shape, nm):
            t = pool.tile(shape, bf, tag=nm, name=nm)
            nc.sync.dma_start(t[:], dram_ap)
            return t

        wq_sb = [load(wq_d[cc], [128, 256], f"wq{cc}") for cc in range(2)]
        wk_sb = [load(wk_d[cc], [128, 256], f"wk{cc}") for cc in range(2)]
        wv_sb = [load(wv_d[cc], [128, 512], f"wv{cc}") for cc in range(2)]
        wo_sb = [[load(wo_d[p, cc], [128, 128], f"wo{p}_{cc}") for cc in range(2)]
                 for p in range(4)]
        xT = [load(xT_dram[cc], [128, NTOK], f"xT{cc}") for cc in range(2)]

        # ---- v_aug token tiles (per-head [Wv_h | pad] -> [v_h | 1 | 0]) ----
        vaug = []
        for t_i in range(NTOK // 128):
            vt = pool.tile([128, 512], bf, tag=f"vaug{t_i}", name=f"vaug{t_i}")
            ps = ps_o.tile([128, 512], f32, tag="ps_misc", name="ps_misc")
            for cc in range(2):
                nc.tensor.matmul(
                    ps[:], xT[cc][:, t_i * 128:(t_i + 1) * 128], wv_sb[cc][:],
                    start=(cc == 0), stop=(cc == 1))
            nc.vector.tensor_copy(vt[:], ps[:])
            nc.vector.memset(vt[:].rearrange("p (h t) -> p h t", t=64)[:, :, 32], 1.0)
            vaug.append(vt)

        # ---- attention main loops ----
        dn = pool.tile([128, AVW], bf, tag="dn", name="dn")        # compacted denominators (bf16)
        rec = pool.tile([128, AVW], bf, tag="rec", name="rec")      # their reciprocals
        rec_d = nc.dram_tensor(f"rec_dram_{tag}", [128, AVW], bf)   # DRAM bounce for partition-bcast
        pairbufs = {}

        GRP = 4 if not is_width else 2       # blocks per projection group (512 tokens)
        for hf in range(2):
            blks = list(range(hf * half, (hf + 1) * half))
            qk_groups = {}
            for blk in blks:
                # --- grouped q/k projection: [q|k][ic] over GRP blocks ---
                g = blk // GRP
                if g not in qk_groups:
                    gtiles = []
                    for ti, w_sb in ((0, wq_sb), (1, wk_sb)):
                        gt = work.tile([128, 1024], bf, tag=f"qkg{ti}", name=f"qkg{ti}")
                        for ic in range(2):
                            psqk = ps_o.tile([128, 512], f32, tag="ps_misc", name="ps_misc")
                            for cc in range(2):
                                nc.tensor.matmul(
                                    psqk[:],
                                    w_sb[cc][:, ic * 128:(ic + 1) * 128],
                                    xT[cc][:, g * 512:(g + 1) * 512],
                                    start=(cc == 0), stop=(cc == 1))
                            nc.vector.tensor_copy(gt[:, ic * 512:(ic + 1) * 512], psqk[:])
                        gtiles.append(gt)
                    qk_groups = {g: gtiles}      # keep only current group
                qg, kg = qk_groups[g]
                boff = (blk % GRP) * 128 if not is_width else (blk % GRP) * 256
                if not is_width:
                    aT = at_pool.tile([128, 8 * 128], bf, tag="aT", name="aT")
                    ps = ps_s.tile([128, 2048], f32, tag="s_ps", name="s_ps")
                    for h in range(8):
                        th, hh = divmod(h, 4)
                        col = 512 * hh + 128 * th        # bank = row-group
                        nc.tensor.matmul(
                            ps[:, col:col + 128],
                            kg[hh * 32:(hh + 1) * 32, th * 512 + boff: th * 512 + boff + 128],
                            qg[hh * 32:(hh + 1) * 32, th * 512 + boff: th * 512 + boff + 128],
                            start=True, stop=True,
                            tile_position=(hh * 32, 0))
                    # aT col for head h=4*th+hh is 128*h = 512*th + 128*hh
                    nc.scalar.activation(
                        aT[:].rearrange("p (th hh x) -> p hh th x", th=2, hh=4),
                        ps[:].rearrange("p (hh b) -> p hh b", hh=4)[:, :, 0:256]
                             .rearrange("p hh (th x) -> p hh th x", th=2),
                        Exp, scale=SCALE)
                else:
                    aT = at_pool.tile([128, 2 * 8 * 256], bf, tag="aT", name="aT")
                    for yc in range(2):
                        ps = ps_s.tile([128, 2048], f32, tag="s_ps", name="s_ps")
                        for h in range(8):
                            th, hh = divmod(h, 4)
                            col = 512 * hh + 256 * th    # bank = row-group
                            nc.tensor.matmul(
                                ps[:, col:col + 256],
                                kg[hh * 32:(hh + 1) * 32, th * 512 + boff + yc * 128: th * 512 + boff + (yc + 1) * 128],
                                qg[hh * 32:(hh + 1) * 32, th * 512 + boff: th * 512 + boff + 256],
                                start=True, stop=True,
                                tile_position=(hh * 32, 0))
                        nc.scalar.activation(
                            aT[:, yc * 2048:(yc + 1) * 2048].rearrange(
                                "p (th hh x) -> p hh th x", th=2, hh=4),
                            ps[:].rearrange("p (hh b) -> p hh b", hh=4)[:, :, 0:512]
                                 .rearrange("p hh (th x) -> p hh th x", th=2),
                            Exp, scale=SCALE)

                # --- AV with denominator column, 2-head col packing per pair ---
                av = ps_av.tile([128, AVW], f32, tag="av_ps", name="av_ps")
                for p in range(4):
                    osl = slice(p * XBLK, (p + 1) * XBLK)
                    for s in range(2):
                        h = 2 * p + s
                        op = 64 * s
                        if not is_width:
                            nc.tensor.matmul(
                                av[op:op + 64, osl],
                                vaug[blk][:, h * 64:(h + 1) * 64],
                                aT[:, h * 128:(h + 1) * 128],
                                start=True, stop=True,
                                tile_position=(0, op))
                        else:
                            for yc in range(2):
                                nc.tensor.matmul(
                                    av[op:op + 64, osl],
                                    vaug[2 * blk + yc][:, h * 64:(h + 1) * 64],
                                    aT[:, yc * 2048 + h * 256:yc * 2048 + (h + 1) * 256],
                                    start=(yc == 0), stop=(yc == 1),
                                    tile_position=(0, op))

                # --- stash unnormalized pair block; compact denom rows from it ---
                r0 = hf * 2 * half + 2 * (blk % half)
                pb = pb_pool.tile([128, AVW], bf, tag="pairbuf", name="pairbuf")
                nc.vector.tensor_copy(pb[:], av[:])
                nc.scalar.dma_start(dn[r0:r0 + 1, :], pb[32:33, :])
                nc.scalar.dma_start(dn[r0 + 1:r0 + 2, :], pb[96:97, :])
                pairbufs[blk] = pb

            # --- reciprocal for this half ---
            rsl = slice(hf * 2 * half, (hf + 1) * 2 * half)
            with nc.allow_low_precision(reason="bf16 softmax denominators"):
                nc.vector.reciprocal(rec[rsl, :], dn[rsl, :])
            nc.sync.dma_start(rec_d[rsl, :], rec[rsl, :])

            # --- normalize into 4-block pbn buffer + batched output projection ---
            OG = 4 if not is_width else 2        # blocks per oproj group (512 tokens)
            pbn = None
            for blk in blks:
                r0 = hf * 2 * half + 2 * (blk % half)
                bc = work.tile([128, AVW], bf, tag="bc", name="bc")
                nc.scalar.dma_start(
                    bc[0:64, :],
                    rec_d[r0:r0 + 1, :].partition_broadcast(64).rearrange("p o f -> p (o f)"))
                nc.scalar.dma_start(
                    bc[64:128, :],
                    rec_d[r0 + 1:r0 + 2, :].partition_broadcast(64).rearrange("p o f -> p (o f)"))
                gi = blk % OG
                if gi == 0:
                    pbn = work.tile([128, 4 * 512], bf, tag="pairbufN", name="pairbufN")
                # pbn layout: [128, (p:4) (gi:OG) (x:XBLK)] so pair p spans 512 tokens
                pv = pbn[:].rearrange("q (p g x) -> q p g x", p=4, g=OG)
                nc.gpsimd.tensor_tensor(
                    pv[:, :, gi, :],
                    pairbufs[blk][:].rearrange("q (p x) -> q p x", p=4),
                    bc[:].rearrange("q (p x) -> q p x", p=4), mult)
                del pairbufs[blk]
                if gi == OG - 1:
                    g0 = (blk // OG) * OG        # first block of group
                    for cc in range(2):
                        po = ps_o.tile([128, 512], f32, tag="ps_misc", name="ps_misc")
                        for p in range(4):
                            nc.tensor.matmul(
                                po[:],
                                wo_sb[p][cc][:],
                                pbn[:, p * 512:(p + 1) * 512],
                                start=(p == 0), stop=(p == 3))
                        osb = work.tile([128, 512], f32, tag="osb", name="osb")
                        nc.vector.tensor_copy(osb[:], po[:])
                        nc.sync.dma_start(
                            out_d[cc][:, g0 * XBLK:g0 * XBLK + 512], osb[:])

    with TileContext(nc) as tc:
        with contextlib.ExitStack() as c1:
            phase(tc, c1, xh, wts["wq_h"], wts["wk_h"], wts["wv_h"], wts["wo_h"],
                  out_h, is_width=False)
        with contextlib.ExitStack() as c2:
            phase(tc, c2, xw, wts["wq_w"], wts["wk_w"], wts["wv_w"], wts["wo_w"],
                  out_w, is_width=True)

    nc.compile()
    return nc


def _prep_weights(inp):
    """Host-side weight layouts, bf16."""
    def chunks(Wm):                      # [256, 256] -> [2, 128, 256] (lhsT chunks)
        return np.ascontiguousarray(Wm.reshape(2, 128, 256)).astype(BF16)

    def v_pad(Wm):                       # -> [2, 128, 8*64]: per-head [Wv_h | 0...]
        out = np.zeros((2, 128, 512), np.float32)
        for hh in range(8):
            out[:, :, hh * 64:hh * 64 + 32] = Wm.reshape(2, 128, 256)[:, :, hh * 32:(hh + 1) * 32]
        return out.astype(BF16)

    def wo_aug(Wo):                      # -> [4 pairs, 2 cc, 128 K(padded), 128 M]
        out = np.zeros((4, 2, 128, 128), np.float32)
        for p in range(4):
            for cc in range(2):
                out[p, cc, 0:32, :] = Wo[64 * p:64 * p + 32, cc * 128:(cc + 1) * 128]
                out[p, cc, 64:96, :] = Wo[64 * p + 32:64 * p + 64, cc * 128:(cc + 1) * 128]
        return out.astype(BF16)

    d = {}
    for ph in ("h", "w"):
        d[f"wq_{ph}"] = chunks(np.asarray(inp[f"Wq_{ph}"], np.float32))
        d[f"wk_{ph}"] = chunks(np.asarray(inp[f"Wk_{ph}"], np.float32))
        d[f"wv_{ph}"] = v_pad(np.asarray(inp[f"Wv_{ph}"], np.float32))
        d[f"wo_{ph}"] = wo_aug(np.asarray(inp[f"Wo_{ph}"], np.float32))
    return d


def kernel(x, Wq_h, Wk_h, Wv_h, Wo_h, bo_h, Wq_w, Wk_w, Wv_w, Wo_w, bo_w, h, w,
           _trace=False):
    from concourse.bass_utils import run_bass_kernel_spmd

    x = np.asarray(x, np.float32)
    xs = x.reshape(B, H, W, C)
    wd = _prep_weights(dict(Wq_h=Wq_h, Wk_h=Wk_h, Wv_h=Wv_h, Wo_h=Wo_h,
                            Wq_w=Wq_w, Wk_w=Wk_w, Wv_w=Wv_w, Wo_w=Wo_w))

    in_maps = []
    for core in range(8):
        b, j = divmod(core, 4)
        xh_a = xs[b][:, j * WC:(j + 1) * WC, :].transpose(2, 1, 0)   # [C, Wc, H]
        xw_a = xs[b][j * HC:(j + 1) * HC, :, :].transpose(2, 0, 1)   # [C, Hc, W]
        m = dict(wd)
        m["xh"] = np.ascontiguousarray(xh_a).reshape(2, 128, NTOK).astype(BF16)
        m["xw"] = np.ascontiguousarray(xw_a).reshape(2, 128, NTOK).astype(BF16)
        in_maps.append(m)

    if "nc" not in _compiled:
        _compiled["nc"] = _build_module()
    nc = _compiled["nc"]

    kw = {}
    if _trace:
        kw = dict(trace=True, trace_cores=[0])
    res = run_bass_kernel_spmd(nc, in_maps, core_ids=list(range(8)), **kw)
    _compiled["last_result"] = res

    out = np.zeros((B, H, W, C), np.float32)
    for core in range(8):
        b, j = divmod(core, 4)
        oh = np.asarray(res.results[core]["out_h"])   # [2(cc), 128(ci), WC*128(n)]
        ow = np.asarray(res.results[core]["out_w"])   # [2(cc), 128(ci), HC*256(n)]
        # outT[c, n], c = cc*128 + ci; height n = w*128 + r -> [r, w, c]
        oh_t = oh.reshape(256, WC, 128).transpose(2, 1, 0)
        out[b, :, j * WC:(j + 1) * WC, :] += oh_t
        # width n = r*256 + wcol -> [r, wcol, c]
        ow_t = ow.reshape(256, HC, 256).transpose(1, 2, 0)
        out[b, j * HC:(j + 1) * HC, :, :] += ow_t
    out += np.asarray(bo_h, np.float32) + np.asarray(bo_w, np.float32)
    return out.reshape(B, H * W, C)

